# revision 24
# baseline (speedup 1.0000x reference)
"""MultiHeadSelfAttention + residual + LayerNorm on 8 TRN2 NeuronCores.

Sharding: 2 cores per batch element (B=4), heads split 8/8 within the pair
(tensor parallel). Attention runs as one flat software-pipelined stream over
all (chunk, head-pair, key-tile) steps with scores emitted TWO steps ahead,
so the ACT engine (the structural bottleneck: 33.5M softmax-exp elements per
core) streams batched [128,1024] two-head exps back-to-back. PV matmuls run
in each exp's shadow; the [V|ones] fold accumulates softmax denominators for
free.

v2 over the AR baseline (556us):
- Row-sharded out-proj partials are combined with pairwise ReduceScatter
  (half the wire bytes of AllReduce); each core then LayerNorms ONLY its own
  128-row shard of each 256-row block (half the DVE work). The rank
  asymmetry (which rows a core owns) is routed through host-prepared
  per-core tensors: `xr` carries the residual x rows for this core's shards
  and the output y is the compact [1024, D] of owned rows that the host
  scatters back.
- The softmax denominator reciprocal no longer does a 4-DMA DRAM bounce:
  gpsimd.partition_broadcast fans the den row across 64 partitions, then a
  DVE reciprocal + multiply normalize in-SBUF. (SBUF-source broadcast DMA is
  illegal; gpsimd is otherwise idle so the broadcast is free.)
- LayerNorm input DMAs (rs_out shard + xr residual) are issued at
  ReduceScatter-fire time, several microseconds before the LN slot, so the
  in-order DVE queue never head-of-line blocks on a late DMA (the 8us/
  boundary stall pattern of the baseline: LN ADD waiting on its residual
  DMA blocked the epilogue acc copies that the out-proj PSUM-bank WAR
  needed, idling the PE long enough for a HAM re-throttle).

LayerNorm's 1/std runs on DVE via Newton iteration from a reciprocal seed,
keeping ACT exp-only: one activation-table load for the whole kernel.
(Measured dead ends, do not revisit without 2 samples each: Sqrt or ln/exp
rstd on ACT -> table churn; LN emitted near its AR -> in-order queue stalls;
ACT accum_out stats -> cross-engine ping-pong; keep-warm dummy matmuls;
Scalar-queue weight loads; interleaved h0/h1 denominator DMA chains.)

Phase A emits chunk sc+1's transposes before chunk sc's QKV matmuls (no PE
gaps -> no HAM re-throttle).

Self-contained: shapes/sharding hardcoded; builds and caches the NEFF on
first call. bo/2 is folded into each core's out-proj partial host-side.
"""
import numpy as np

import concourse.bass as bass
import concourse.tile as tile
from concourse import bacc, mybir
from concourse.bass_utils import run_bass_kernel_spmd
from concourse.masks import make_identity

F32 = mybir.dt.float32
F32R = mybir.dt.float32r
BF16 = mybir.dt.bfloat16

B, S, D, H, DEPTH = 4, 2048, 1024, 16, 64
HL = 8            # heads per core
EL = 512          # local e width (HL * DEPTH)
CT = 8            # c tiles (D / 128)
ST = 16           # s tiles (S / 128)
SC = 4            # s chunks (S / 512)
ET = 4            # local e tiles (EL / 128)
QC = 4            # query chunks in phase B (S / 512)
NBLK = 8          # 256-row reduce-scatter blocks
EPS = 1e-6
RG = [[0, 1], [2, 3], [4, 5], [6, 7]]

_CACHE = {}
_LAST_IN_MAPS = None


def _build():
    nc = bacc.Bacc("TRN2", target_bir_lowering=False, debug=False, num_devices=8)

    x_in = nc.dram_tensor("x", [S, D], F32R, kind="ExternalInput")
    xr_in = nc.dram_tensor("xr", [NBLK * 128, D], F32, kind="ExternalInput")
    wq_in = nc.dram_tensor("wq", [D, EL], F32R, kind="ExternalInput")
    wk_in = nc.dram_tensor("wk", [D, EL], F32R, kind="ExternalInput")
    wv_in = nc.dram_tensor("wv", [D, EL], F32R, kind="ExternalInput")
    wo_in = nc.dram_tensor("wo", [EL, D], F32R, kind="ExternalInput")
    bqk_in = nc.dram_tensor("bqk", [128, 2 * ET], F32, kind="ExternalInput")
    bv_in = nc.dram_tensor("bv", [1, EL], F32, kind="ExternalInput")
    bo_in = nc.dram_tensor("bo", [1, D], F32, kind="ExternalInput")
    gamma_in = nc.dram_tensor("gamma", [1, D], F32, kind="ExternalInput")
    beta_in = nc.dram_tensor("beta", [1, D], F32, kind="ExternalInput")
    y_out = nc.dram_tensor("y", [NBLK * 128, D], F32, kind="ExternalOutput")

    with tile.TileContext(nc) as tc:
        with tc.tile_pool(name="const", bufs=1) as const, \
             tc.tile_pool(name="dram", bufs=1, space="DRAM") as dram:

            # NOTE (measured): the runtime staggers pair-core launches by
            # ~60us and exec time is per-core span, so an entry barrier only
            # moves the skew to the front of core 0's span (563us vs 529us
            # without); collectives completing ~60us after this core fires
            # them must instead be TOLERATED by consuming their results as
            # late as possible on queues nothing else shares.
            ident_f = const.tile([128, 128], F32)
            make_identity(nc, ident_f[:])
            ident = const.tile([128, 128], F32R)
            nc.vector.tensor_copy(ident[:], ident_f[:])
            ones1 = const.tile([128, 1], F32)
            nc.gpsimd.memset(ones1[:], 1.0)

            # ReduceScatter blocks of 256 rows; bf16 payload halves
            # collective bytes; each core keeps only its own 128-row shard.
            y_part = [dram.tile([256, D], BF16, name=f"y_part{i}")
                      for i in range(NBLK)]
            rs_out = [dram.tile([128, D], BF16, name=f"rs_out{i}")
                      for i in range(NBLK)]
            den_d = dram.tile([QC * ET * 2, 1, 512], F32)
            rec_d = dram.tile([QC * ET * 2, 64, 8], F32)

            bqk_sb = const.tile([128, 2 * ET], F32)
            nc.sync.dma_start(bqk_sb[:], bqk_in.ap()[:])
            bv_bc = const.tile([128, EL], F32)
            nc.sync.dma_start(bv_bc[:], bv_in.ap().to_broadcast((128, EL)))

            with tc.tile_pool(name="qkv", bufs=1) as qkvp:
                kt = qkvp.tile([128, ET, S], F32R)                 # K^T [e, s]
                qt = qkvp.tile([128, ET, S], F32R)                 # Q^T [e, s]
                vt = qkvp.tile([128, ST, HL, DEPTH + 1], F32R)     # V natural + ones
                nc.vector.tensor_copy(vt[:, :, :, DEPTH:DEPTH + 1],
                                      ones1[:].to_broadcast((128, ST, HL, 1)))

                # ---- phase A: transpose X per chunk; project Q, K, V.
                # Transposes for chunk sc+1 are emitted BEFORE chunk sc's
                # matmuls (transpose-ahead) so the PE never waits on the
                # transpose->DVE-copy round trip at chunk boundaries; 4
                # transposes share one PSUM bank and drain in one strided
                # DVE copy. x-chunk DMAs precede the weight DMAs so the
                # first transposes start immediately.
                with tc.tile_pool(name="xnA", bufs=4) as xnA, \
                     tc.tile_pool(name="xtA", bufs=2) as xtA, \
                     tc.tile_pool(name="w3", bufs=1) as w3, \
                     tc.tile_pool(name="tpA", bufs=2, space="PSUM") as tpA, \
                     tc.tile_pool(name="psA", bufs=4, space="PSUM") as psA:

                    def emit_transposes(sc):
                        xt_c = xtA.tile([128, CT, 512], F32R, name="xt_c", tag="xt_c")
                        for sl in range(4):
                            si = 4 * sc + sl
                            xn = xnA.tile([128, D], F32R, name="xn", tag="xn")
                            nc.sync.dma_start(xn[:], x_in.ap()[128 * si:128 * (si + 1), :])
                            for g in range(2):
                                tp4 = tpA.tile([128, 512], F32R, name="tp4", tag="tp4")
                                for c4 in range(4):
                                    ci = 4 * g + c4
                                    nc.tensor.transpose(
                                        tp4[:, 128 * c4:128 * (c4 + 1)],
                                        xn[:, 128 * ci:128 * (ci + 1)], ident[:])
                                nc.vector.tensor_copy(
                                    xt_c[:, 4 * g:4 * (g + 1), 128 * sl:128 * (sl + 1)],
                                    tp4[:].rearrange("p (a b) -> p a b", a=4))
                        return xt_c

                    def emit_w_loads():
                        wsb = {}
                        for nm, wdram in (("q", wq_in), ("k", wk_in), ("v", wv_in)):
                            wsb[nm] = w3.tile([128, CT, EL], F32R, name=f"w{nm}")
                            for ci in range(CT):
                                nc.sync.dma_start(wsb[nm][:, ci, :],
                                                  wdram.ap()[128 * ci:128 * (ci + 1), :])
                        return wsb

                    xt_tiles = {0: emit_transposes(0)}
                    wsb = None
                    for sc in range(SC):
                        if sc + 1 < SC:
                            xt_tiles[sc + 1] = emit_transposes(sc + 1)
                        if wsb is None:
                            wsb = emit_w_loads()
                        cs = slice(512 * sc, 512 * (sc + 1))
                        xt_c = xt_tiles.pop(sc)
                        for dst, wname, bcol in ((qt, "q", 0), (kt, "k", ET)):
                            for j in range(ET):
                                ps = psA.tile([128, 512], F32, name="pqk", tag="pqk")
                                for ci in range(CT):
                                    nc.tensor.matmul(
                                        ps[:], wsb[wname][:, ci, 128 * j:128 * (j + 1)],
                                        xt_c[:, ci, :], start=(ci == 0), stop=(ci == CT - 1))
                                nc.vector.tensor_scalar_add(
                                    dst[:, j, cs], ps[:], bqk_sb[:, bcol + j:bcol + j + 1])
                        for sl in range(4):
                            si = 4 * sc + sl
                            ps = psA.tile([128, 512], F32, name="pv", tag="pqk")
                            for ci in range(CT):
                                nc.tensor.matmul(
                                    ps[:], xt_c[:, ci, 128 * sl:128 * (sl + 1)],
                                    wsb["v"][:, ci, :], start=(ci == 0), stop=(ci == CT - 1))
                            nc.vector.tensor_add(
                                vt[:, si, :, 0:DEPTH],
                                ps[:].rearrange("p (h e) -> p h e", h=HL),
                                bv_bc[:].rearrange("p (h e) -> p h e", h=HL))

                # ---- phase B: attention; out-proj/RS/LN pipelined into it ----
                with tc.tile_pool(name="wo", bufs=1) as wop, \
                     tc.tile_pool(name="lnc", bufs=1) as lnc, \
                     tc.tile_pool(name="atc", bufs=2) as atcp, \
                     tc.tile_pool(name="ep3", bufs=2) as ep3, \
                     tc.tile_pool(name="psb", bufs=2) as psb, \
                     tc.tile_pool(name="ysb", bufs=2) as ysb, \
                     tc.tile_pool(name="lni", bufs=3) as lni, \
                     tc.tile_pool(name="ln", bufs=2) as ln, \
                     tc.tile_pool(name="sps", bufs=3, space="PSUM") as sps, \
                     tc.tile_pool(name="aps", bufs=1, space="PSUM") as aps:
                    wo_sb = wop.tile([128, ET, D], F32R)
                    for j in range(ET):
                        nc.sync.dma_start(wo_sb[:, j, :], wo_in.ap()[128 * j:128 * (j + 1), :])
                    bo_bc = lnc.tile([128, D], F32)
                    nc.sync.dma_start(bo_bc[:], bo_in.ap().to_broadcast((128, D)))
                    gam_bc = lnc.tile([128, D], F32)
                    nc.sync.dma_start(gam_bc[:], gamma_in.ap().to_broadcast((128, D)))
                    bet_bc = lnc.tile([128, D], F32)
                    nc.sync.dma_start(bet_bc[:], beta_in.ap().to_broadcast((128, D)))

                    def outproj_group(qc, stl, mh, a_prev):
                        """One [128 rows x 512 cols] out-proj partial for chunk qc."""
                        rloc = slice(128 * stl, 128 * (stl + 1))
                        ms = slice(512 * mh, 512 * (mh + 1))
                        ps = aps.tile([128, 512], F32, name="py",
                                      tag=f"acc{mh}")
                        for j in range(ET):
                            nc.tensor.matmul(ps[:], a_prev[:, j, rloc],
                                             wo_sb[:, j, ms],
                                             start=(j == 0), stop=(j == ET - 1))
                        y_sb = ysb.tile([128, 512], BF16, name="y_sb", tag="y_sb")
                        # bo/2 folded here (host halves bo) so LN skips its add
                        nc.vector.tensor_add(y_sb[:], ps[:], bo_bc[:, ms])
                        grow = 512 * qc + 128 * stl
                        bi = grow // 256
                        nc.sync.dma_start(
                            y_part[bi][grow - 256 * bi:grow - 256 * bi + 128, ms],
                            y_sb[:])

                    # rs_block only fires the collective; the LN input DMAs
                    # are issued by prefetch() at a step where the RS wire is
                    # provably complete, so neither the in-order sync queue
                    # (HOL on the RS-done wait) nor the DVE queue (LN add
                    # waiting on a late DMA) ever blocks on the collective.
                    ln_in = {}

                    def rs_block(bi):
                        nc.gpsimd.collective_compute(
                            "ReduceScatter", mybir.AluOpType.add,
                            replica_groups=RG,
                            ins=[y_part[bi].opt()], outs=[rs_out[bi].opt()])

                    def prefetch(bi):
                        tb = lni.tile([128, D], BF16, name="tb", tag="tb")
                        nc.sync.dma_start(tb[:], rs_out[bi][:, :])
                        r = lni.tile([128, D], F32, name="r", tag="r")
                        nc.sync.dma_start(
                            r[:], xr_in.ap()[128 * bi:128 * (bi + 1), :])
                        ln_in[bi] = (tb, r)

                    def ln_block(bi, use_act=False):
                        """Residual + LayerNorm for this core's 128-row shard
                        of block bi. bo/2 pre-folded into the partials."""
                        tb, r = ln_in.pop(bi)
                        t = ln.tile([128, D], F32, name="t", tag="t")
                        nc.vector.tensor_add(t[:], r[:], tb[:])
                        vv = ln.tile([128, 1], F32, name="vv", tag="vv")
                        # stats stay on DVE: an ACT accum_out variant was
                        # tried and regressed -- the per-tile ACT<->DVE
                        # ping-pong costs more in cross-engine semaphore
                        # latency than the bn_stats work saved
                        stats = ln.tile([128, 2, 6], F32, name="stats",
                                        tag="stats")
                        tv = t[:].rearrange("p (a b) -> p a b", a=2)
                        for sub in range(2):
                            nc.vector.bn_stats(stats[:, sub, :],
                                               tv[:, sub, :])
                        mv = ln.tile([128, 2], F32, name="mv", tag="mv")
                        nc.vector.bn_aggr(mv[:], stats[:])
                        mean_ap = mv[:, 0:1]
                        nc.vector.tensor_scalar_add(vv[:], mv[:, 1:2], EPS)
                        # rstd = 1/sqrt(var+eps) on DVE (Newton from 1/v
                        # seed) so ACT runs exp only -> one table load ever
                        rstd = ln.tile([128, 1], F32, name="rstd", tag="rstd")
                        nc.vector.reciprocal(rstd[:], vv[:])
                        yt = ln.tile([128, 1], F32, name="yt", tag="yt")
                        for _ in range(2):
                            nc.vector.tensor_mul(yt[:], rstd[:], rstd[:])
                            nc.vector.tensor_mul(yt[:], yt[:], vv[:])
                            nc.vector.tensor_scalar(
                                yt[:], yt[:], -0.5, 1.5,
                                mybir.AluOpType.mult, mybir.AluOpType.add)
                            nc.vector.tensor_mul(rstd[:], rstd[:], yt[:])
                        o = t  # in-place: (t-mean)*rstd overwrites t (SBUF)
                        if use_act:
                            # drain only: apply (t-mean)*rstd on the (by
                            # then idle) ACT engine as
                            # Identity(rstd*t + (-mean*rstd)); Identity is
                            # in exp's table set so no reload. In-loop
                            # blocks keep this on DVE -- ACT is the phase
                            # B bottleneck.
                            nb = ln.tile([128, 1], F32, name="nb", tag="nb")
                            nc.vector.tensor_scalar(
                                nb[:], mean_ap, rstd[:], -1.0,
                                mybir.AluOpType.mult, mybir.AluOpType.mult)
                            nc.scalar.activation(
                                o[:], t[:],
                                mybir.ActivationFunctionType.Identity,
                                bias=nb[:], scale=rstd[:])
                        else:
                            nc.vector.tensor_scalar(
                                o[:], t[:], mean_ap, rstd[:],
                                mybir.AluOpType.subtract, mybir.AluOpType.mult)
                        nc.vector.tensor_mul(o[:], o[:], gam_bc[:])
                        nc.vector.tensor_add(o[:], o[:], bet_bc[:])
                        nc.sync.dma_start(
                            y_out.ap()[128 * bi:128 * (bi + 1), :], o[:])

                    # Flat software pipeline over all (qc, j, kti) steps with
                    # scores emitted TWO steps ahead: S(i+1) completes a full
                    # exp-period before exp(i+1) needs it, so the exp stream
                    # never waits on a PE round trip (was ~200ns/exp). PV(i-1)
                    # runs in exp(i)'s shadow. sps triple-buffers (6 banks);
                    # out-proj groups time-share the accumulator banks at
                    # j-boundaries (epilogue has just released them).
                    steps = [(qc, j, kti)
                             for qc in range(QC) for j in range(ET)
                             for kti in range(ST)]
                    sp_t, pp_t, accs_map, a_map = {}, {}, {}, {}

                    def emit_scores(i):
                        if i >= len(steps):
                            return
                        sqc, sj, skti = steps[i]
                        ks = slice(128 * skti, 128 * (skti + 1))
                        sqs = slice(512 * sqc, 512 * (sqc + 1))
                        sp = sps.tile([128, 1024], F32, name="sp", tag="sp")
                        for h01 in range(2):
                            rows = slice(64 * h01, 64 * (h01 + 1))
                            nc.tensor.matmul(sp[:, 512 * h01:512 * (h01 + 1)],
                                             kt[rows, sj, ks], qt[rows, sj, sqs],
                                             start=True, stop=True)
                        sp_t[i] = sp

                    def emit_pv(i):
                        pqc, pj, pkti = steps[i]
                        if pkti == 0:
                            accs_map[(pqc, pj)] = [
                                aps.tile([DEPTH + 1, 512], F32, name=f"acc{h}",
                                         tag=f"acc{h}")
                                for h in range(2)]
                        accs = accs_map[(pqc, pj)]
                        pp = pp_t.pop(i)
                        for h01 in range(2):
                            nc.tensor.matmul(accs[h01][:],
                                             vt[:, pkti, 2 * pj + h01, :],
                                             pp[:, 512 * h01:512 * (h01 + 1)],
                                             start=(pkti == 0),
                                             stop=(pkti == ST - 1))

                    def emit_epilogue(qc, j, a_t):
                        accs = accs_map.pop((qc, j))
                        # both acc-release copies first: frees both PSUM
                        # accumulator banks (out-proj groups wait on them via
                        # tag WAR) before the slow denominator DMA round
                        # trips, which then overlap h0/h1. The DRAM bounce is
                        # deliberate: a [1,512] DVE reciprocal costs 3.2us on
                        # one lane; bounced to a [64,8] scatter it costs 160ns
                        # and the DVE queue stays free.
                        acc_sbs = []
                        for h01 in range(2):
                            acc_sb = ep3.tile([DEPTH + 1, 512], F32, name="acc_sb",
                                              tag="acc_sb")
                            nc.vector.tensor_copy(acc_sb[:], accs[h01][:])
                            acc_sbs.append(acc_sb)
                        for h01 in range(2):
                            idx = (qc * ET + j) * 2 + h01
                            acc_sb = acc_sbs[h01]
                            nc.sync.dma_start(den_d[idx],
                                              acc_sb[DEPTH:DEPTH + 1, :])
                            rin = ep3.tile([64, 8], F32, name="rin", tag="rin")
                            nc.sync.dma_start(rin[:], den_d[idx].rearrange(
                                "a (p f) -> (a p) f", p=64))
                            nc.vector.reciprocal(rin[:], rin[:])
                            nc.sync.dma_start(rec_d[idx], rin[:])
                            rbc = ep3.tile([64, 512], F32, name="rbc", tag="rbc")
                            rsrc = rec_d[idx]
                            nc.sync.dma_start(
                                rbc[:],
                                bass.AP(tensor=rsrc.tensor, offset=rsrc.offset,
                                        ap=[[0, 64], [1, 512]]))
                            if h01 == 0:
                                nc.vector.tensor_mul(a_t[0:64, j, :],
                                                     acc_sb[0:DEPTH, :], rbc[:])
                            else:
                                nrm = ep3.tile([64, 512], F32R, name="nrm", tag="nrm")
                                nc.vector.tensor_mul(nrm[:], acc_sb[0:DEPTH, :],
                                                     rbc[:])
                                nc.sync.dma_start(a_t[64:128, j, :], nrm[:])

                    a_prev = None
                    fired, prefd = [], []   # (block, step index) queues
                    emit_scores(0)
                    emit_scores(1)
                    for i, (qc, j, kti) in enumerate(steps):
                        pp = psb.tile([128, 1024], F32R, name="pp", tag="pp")
                        nc.scalar.activation(pp[:], sp_t.pop(i)[:],
                                             mybir.ActivationFunctionType.Exp,
                                             scale=0.125)
                        pp_t[i] = pp
                        if i >= 1 and (i - 1) in pp_t:
                            emit_pv(i - 1)
                        if i + 2 not in sp_t:
                            emit_scores(i + 2)
                    # staged LN pipeline in quiet mid-loop slots, away
                    # from the boundary's epilogue/out-proj/RS pile-up:
                    # prefetch >=8 steps after the RS fired (wire done),
                    # LN compute >=4 steps after the prefetch (DMA done)
                        # prefetch only when the RS is PROVABLY complete
                        # (fire + pair skew + wire < 55 steps): a tb DMA
                        # waiting on a live collective would head-of-line
                        # block the sync queue and stall the epilogue chain
                        if kti == 8 and fired and i - fired[0][1] >= 55:
                            bi = fired.pop(0)[0]
                            prefetch(bi)
                            prefd.append((bi, i))
                        if kti == 4 and prefd and i - prefd[0][1] >= 6:
                            ln_block(prefd.pop(0)[0])
                        if kti == ST - 1:
                            emit_pv(i)
                            if j == 0:
                                a_map[qc] = atcp.tile([128, ET, 512], F32R,
                                                      name="a_t", tag="a_t")
                            emit_epilogue(qc, j, a_map[qc])
                            # pre-emit the next j's second scores tile so it
                            # sits ahead of the out-proj burst on the PE queue
                            # (same exp(i) gate as PV(i), so no extra stall)
                            if i + 3 < len(steps):
                                emit_scores(i + 3)
                            # boundary pieces: out-proj of qc-1 front-loaded
                            # [3,3,2,0] across the j-boundaries so both RS
                            # blocks fire a half-chunk early and the CC is
                            # idle when the drain's RS6/RS7 arrive
                            if qc >= 1:
                                for gi in ([0, 1, 2], [3, 4, 5], [6, 7], [])[j]:
                                    outproj_group(qc - 1, gi // 2, gi % 2,
                                                  a_prev)
                                if j in (1, 2):
                                    bi = 2 * (qc - 1) + (j - 1)
                                    rs_block(bi)
                                    fired.append((bi, i))
                            if j == ET - 1:
                                a_prev = a_map.pop(qc)
                    # drain: out-proj + RS for the last chunk, then ALL
                    # remaining LNs strictly after the out-proj adds so a
                    # late RS never head-of-line blocks the DVE queue ahead
                    # of out-proj work
                    outproj_group(QC - 1, 0, 0, a_prev)
                    outproj_group(QC - 1, 0, 1, a_prev)
                    outproj_group(QC - 1, 1, 0, a_prev)
                    outproj_group(QC - 1, 1, 1, a_prev)
                    rs_block(6)
                    outproj_group(QC - 1, 2, 0, a_prev)
                    outproj_group(QC - 1, 2, 1, a_prev)
                    outproj_group(QC - 1, 3, 0, a_prev)
                    outproj_group(QC - 1, 3, 1, a_prev)
                    rs_block(7)
                    for bi, _ in list(prefd):
                        ln_block(bi, use_act=True)
                    for bi in fired:
                        prefetch(bi[0])
                        ln_block(bi[0], use_act=True)
                    prefetch(6)
                    ln_block(6, use_act=True)
                    prefetch(7)
                    ln_block(7, use_act=True)

    nc.compile()
    return nc


def kernel(inputs, Wq, bq, Wk, bk, Wv, bv, Wo, bo, gamma, beta):
    if "nc" not in _CACHE:
        _CACHE["nc"] = _build()
    nc = _CACHE["nc"]

    inputs = np.ascontiguousarray(np.asarray(inputs, dtype=np.float32))
    Wq = np.asarray(Wq, np.float32); Wk = np.asarray(Wk, np.float32)
    Wv = np.asarray(Wv, np.float32); Wo = np.asarray(Wo, np.float32)
    bq = np.asarray(bq, np.float32); bk = np.asarray(bk, np.float32)
    bv = np.asarray(bv, np.float32); bo = np.asarray(bo, np.float32)
    gamma = np.asarray(gamma, np.float32); beta = np.asarray(beta, np.float32)

    in_maps = []
    for c in range(8):
        b, hf = c // 2, c % 2
        es = slice(EL * hf, EL * (hf + 1))
        bqk = np.concatenate([bq[es].reshape(ET, 128).T, bk[es].reshape(ET, 128).T],
                             axis=1)
        # this core's owned 128-row shard of each 256-row RS block
        xr = np.concatenate(
            [inputs[b][256 * i + 128 * hf:256 * i + 128 * (hf + 1)]
             for i in range(NBLK)], axis=0)
        in_maps.append({
            "x": inputs[b],
            "xr": np.ascontiguousarray(xr),
            "wq": np.ascontiguousarray(Wq[:, es]),
            "wk": np.ascontiguousarray(Wk[:, es]),
            "wv": np.ascontiguousarray(Wv[:, es]),
            "wo": np.ascontiguousarray(Wo[es, :]),
            "bqk": np.ascontiguousarray(bqk),
            "bv": bv[es].reshape(1, EL).copy(),
            "bo": (bo / 2.0).reshape(1, D).copy(),
            "gamma": gamma.reshape(1, D).copy(),
            "beta": beta.reshape(1, D).copy(),
        })

    global _LAST_IN_MAPS
    _LAST_IN_MAPS = in_maps
    res = run_bass_kernel_spmd(nc, in_maps, core_ids=list(range(8)))

    out = np.empty((B, S, D), dtype=np.float32)
    for c in range(8):
        b, hf = c // 2, c % 2
        yc = res.results[c]["y"]
        for i in range(NBLK):
            out[b, 256 * i + 128 * hf:256 * i + 128 * (hf + 1)] = \
                yc[128 * i:128 * (i + 1)]
    return out


# revision 25
# speedup vs baseline: 1.0191x; 1.0191x over previous
"""MultiHeadSelfAttention + residual + LayerNorm on 8 TRN2 NeuronCores.

Sharding: 2 cores per batch element (B=4), heads split 8/8 within the pair
(tensor parallel). Attention runs as one flat software-pipelined stream over
all (chunk, head-pair, key-tile) steps with scores emitted TWO steps ahead,
so the ACT engine (the structural bottleneck: 33.5M softmax-exp elements per
core) streams batched [128,1024] two-head exps back-to-back. PV matmuls run
in each exp's shadow; the [V|ones] fold accumulates softmax denominators for
free.

v2 over the AR baseline (556us):
- Row-sharded out-proj partials are combined with pairwise ReduceScatter
  (half the wire bytes of AllReduce); each core then LayerNorms ONLY its own
  128-row shard of each 256-row block (half the DVE work). The rank
  asymmetry (which rows a core owns) is routed through host-prepared
  per-core tensors: `xr` carries the residual x rows for this core's shards
  and the output y is the compact [1024, D] of owned rows that the host
  scatters back.
- The softmax denominator reciprocal no longer does a 4-DMA DRAM bounce:
  gpsimd.partition_broadcast fans the den row across 64 partitions, then a
  DVE reciprocal + multiply normalize in-SBUF. (SBUF-source broadcast DMA is
  illegal; gpsimd is otherwise idle so the broadcast is free.)
- LayerNorm input DMAs (rs_out shard + xr residual) are issued at
  ReduceScatter-fire time, several microseconds before the LN slot, so the
  in-order DVE queue never head-of-line blocks on a late DMA (the 8us/
  boundary stall pattern of the baseline: LN ADD waiting on its residual
  DMA blocked the epilogue acc copies that the out-proj PSUM-bank WAR
  needed, idling the PE long enough for a HAM re-throttle).

LayerNorm's 1/std runs on DVE via Newton iteration from a reciprocal seed,
keeping ACT exp-only: one activation-table load for the whole kernel.
(Measured dead ends, do not revisit without 2 samples each: Sqrt or ln/exp
rstd on ACT -> table churn; LN emitted near its AR -> in-order queue stalls;
ACT accum_out stats -> cross-engine ping-pong; keep-warm dummy matmuls;
Scalar-queue weight loads; interleaved h0/h1 denominator DMA chains.)

Phase A emits chunk sc+1's transposes before chunk sc's QKV matmuls (no PE
gaps -> no HAM re-throttle).

Self-contained: shapes/sharding hardcoded; builds and caches the NEFF on
first call. bo/2 is folded into each core's out-proj partial host-side.
"""
import numpy as np

import concourse.bass as bass
import concourse.tile as tile
from concourse import bacc, mybir
from concourse.bass_utils import run_bass_kernel_spmd
from concourse.masks import make_identity

F32 = mybir.dt.float32
F32R = mybir.dt.float32r
BF16 = mybir.dt.bfloat16

B, S, D, H, DEPTH = 4, 2048, 1024, 16, 64
HL = 8            # heads per core
EL = 512          # local e width (HL * DEPTH)
CT = 8            # c tiles (D / 128)
ST = 16           # s tiles (S / 128)
SC = 4            # s chunks (S / 512)
ET = 4            # local e tiles (EL / 128)
QC = 4            # query chunks in phase B (S / 512)
NBLK = 8          # 256-row reduce-scatter blocks
EPS = 1e-6
RG = [[0, 1], [2, 3], [4, 5], [6, 7]]

_CACHE = {}
_LAST_IN_MAPS = None


def _build():
    nc = bacc.Bacc("TRN2", target_bir_lowering=False, debug=False, num_devices=8)

    x_in = nc.dram_tensor("x", [S, D], F32R, kind="ExternalInput")
    xr_in = nc.dram_tensor("xr", [NBLK * 128, D], F32, kind="ExternalInput")
    wq_in = nc.dram_tensor("wq", [D, EL], F32R, kind="ExternalInput")
    wk_in = nc.dram_tensor("wk", [D, EL], F32R, kind="ExternalInput")
    wv_in = nc.dram_tensor("wv", [D, EL], F32R, kind="ExternalInput")
    wo_in = nc.dram_tensor("wo", [EL, D], F32R, kind="ExternalInput")
    bqk_in = nc.dram_tensor("bqk", [128, 2 * ET], F32, kind="ExternalInput")
    bv_in = nc.dram_tensor("bv", [1, EL], F32, kind="ExternalInput")
    bo_in = nc.dram_tensor("bo", [1, D], F32, kind="ExternalInput")
    gamma_in = nc.dram_tensor("gamma", [1, D], F32, kind="ExternalInput")
    beta_in = nc.dram_tensor("beta", [1, D], F32, kind="ExternalInput")
    y_out = nc.dram_tensor("y", [NBLK * 128, D], F32, kind="ExternalOutput")

    with tile.TileContext(nc) as tc:
        with tc.tile_pool(name="const", bufs=1) as const, \
             tc.tile_pool(name="dram", bufs=1, space="DRAM") as dram:

            # NOTE (measured): the runtime staggers pair-core launches by
            # ~60us and exec time is per-core span, so an entry barrier only
            # moves the skew to the front of core 0's span (563us vs 529us
            # without); collectives completing ~60us after this core fires
            # them must instead be TOLERATED by consuming their results as
            # late as possible on queues nothing else shares.
            ident_f = const.tile([128, 128], F32)
            make_identity(nc, ident_f[:])
            ident = const.tile([128, 128], F32R)
            nc.vector.tensor_copy(ident[:], ident_f[:])
            ones1 = const.tile([128, 1], F32)
            nc.gpsimd.memset(ones1[:], 1.0)

            # ReduceScatter blocks of 256 rows; bf16 payload halves
            # collective bytes; each core keeps only its own 128-row shard.
            y_part = [dram.tile([256, D], BF16, name=f"y_part{i}")
                      for i in range(NBLK)]
            rs_out = [dram.tile([128, D], BF16, name=f"rs_out{i}")
                      for i in range(NBLK)]
            den_d = dram.tile([QC * ET * 2, 1, 512], F32)
            rec_d = dram.tile([QC * ET * 2, 64, 8], F32)

            bqk_sb = const.tile([128, 2 * ET], F32)
            nc.sync.dma_start(bqk_sb[:], bqk_in.ap()[:])
            bv_bc = const.tile([128, EL], F32)
            nc.sync.dma_start(bv_bc[:], bv_in.ap().to_broadcast((128, EL)))

            with tc.tile_pool(name="qkv", bufs=1) as qkvp:
                kt = qkvp.tile([128, ET, S], F32R)                 # K^T [e, s]
                qt = qkvp.tile([128, ET, S], F32R)                 # Q^T [e, s]
                vt = qkvp.tile([128, ST, HL, DEPTH + 1], F32R)     # V natural + ones
                nc.vector.tensor_copy(vt[:, :, :, DEPTH:DEPTH + 1],
                                      ones1[:].to_broadcast((128, ST, HL, 1)))

                # ---- phase A: transpose X per chunk; project Q, K, V.
                # Transposes for chunk sc+1 are emitted BEFORE chunk sc's
                # matmuls (transpose-ahead) so the PE never waits on the
                # transpose->DVE-copy round trip at chunk boundaries; 4
                # transposes share one PSUM bank and drain in one strided
                # DVE copy. x-chunk DMAs precede the weight DMAs so the
                # first transposes start immediately.
                with tc.tile_pool(name="xnA", bufs=4) as xnA, \
                     tc.tile_pool(name="xtA", bufs=2) as xtA, \
                     tc.tile_pool(name="w3", bufs=1) as w3, \
                     tc.tile_pool(name="tpA", bufs=2, space="PSUM") as tpA, \
                     tc.tile_pool(name="psA", bufs=4, space="PSUM") as psA:

                    def emit_transposes(sc):
                        xt_c = xtA.tile([128, CT, 512], F32R, name="xt_c", tag="xt_c")
                        for sl in range(4):
                            si = 4 * sc + sl
                            xn = xnA.tile([128, D], F32R, name="xn", tag="xn")
                            nc.sync.dma_start(xn[:], x_in.ap()[128 * si:128 * (si + 1), :])
                            for g in range(2):
                                tp4 = tpA.tile([128, 512], F32R, name="tp4", tag="tp4")
                                for c4 in range(4):
                                    ci = 4 * g + c4
                                    nc.tensor.transpose(
                                        tp4[:, 128 * c4:128 * (c4 + 1)],
                                        xn[:, 128 * ci:128 * (ci + 1)], ident[:])
                                nc.vector.tensor_copy(
                                    xt_c[:, 4 * g:4 * (g + 1), 128 * sl:128 * (sl + 1)],
                                    tp4[:].rearrange("p (a b) -> p a b", a=4))
                        return xt_c

                    def emit_w_loads():
                        wsb = {}
                        for nm, wdram in (("q", wq_in), ("k", wk_in), ("v", wv_in)):
                            wsb[nm] = w3.tile([128, CT, EL], F32R, name=f"w{nm}")
                            for ci in range(CT):
                                nc.sync.dma_start(wsb[nm][:, ci, :],
                                                  wdram.ap()[128 * ci:128 * (ci + 1), :])
                        return wsb

                    xt_tiles = {0: emit_transposes(0)}
                    wsb = None
                    for sc in range(SC):
                        if sc + 1 < SC:
                            xt_tiles[sc + 1] = emit_transposes(sc + 1)
                        if wsb is None:
                            wsb = emit_w_loads()
                        cs = slice(512 * sc, 512 * (sc + 1))
                        xt_c = xt_tiles.pop(sc)
                        for dst, wname, bcol in ((qt, "q", 0), (kt, "k", ET)):
                            for j in range(ET):
                                ps = psA.tile([128, 512], F32, name="pqk", tag="pqk")
                                for ci in range(CT):
                                    nc.tensor.matmul(
                                        ps[:], wsb[wname][:, ci, 128 * j:128 * (j + 1)],
                                        xt_c[:, ci, :], start=(ci == 0), stop=(ci == CT - 1))
                                nc.vector.tensor_scalar_add(
                                    dst[:, j, cs], ps[:], bqk_sb[:, bcol + j:bcol + j + 1])
                        for sl in range(4):
                            si = 4 * sc + sl
                            ps = psA.tile([128, 512], F32, name="pv", tag="pqk")
                            for ci in range(CT):
                                nc.tensor.matmul(
                                    ps[:], xt_c[:, ci, 128 * sl:128 * (sl + 1)],
                                    wsb["v"][:, ci, :], start=(ci == 0), stop=(ci == CT - 1))
                            nc.vector.tensor_add(
                                vt[:, si, :, 0:DEPTH],
                                ps[:].rearrange("p (h e) -> p h e", h=HL),
                                bv_bc[:].rearrange("p (h e) -> p h e", h=HL))

                # ---- phase B: attention; out-proj/RS/LN pipelined into it ----
                with tc.tile_pool(name="wo", bufs=1) as wop, \
                     tc.tile_pool(name="lnc", bufs=1) as lnc, \
                     tc.tile_pool(name="atc", bufs=2) as atcp, \
                     tc.tile_pool(name="ep3", bufs=2) as ep3, \
                     tc.tile_pool(name="psb", bufs=2) as psb, \
                     tc.tile_pool(name="ysb", bufs=2) as ysb, \
                     tc.tile_pool(name="lni", bufs=3) as lni, \
                     tc.tile_pool(name="ln", bufs=2) as ln, \
                     tc.tile_pool(name="sps", bufs=3, space="PSUM") as sps, \
                     tc.tile_pool(name="aps", bufs=1, space="PSUM") as aps:
                    wo_sb = wop.tile([128, ET, D], F32R)
                    for j in range(ET):
                        nc.sync.dma_start(wo_sb[:, j, :], wo_in.ap()[128 * j:128 * (j + 1), :])
                    bo_bc = lnc.tile([128, D], F32)
                    nc.sync.dma_start(bo_bc[:], bo_in.ap().to_broadcast((128, D)))
                    gam_bc = lnc.tile([128, D], F32)
                    nc.sync.dma_start(gam_bc[:], gamma_in.ap().to_broadcast((128, D)))
                    bet_bc = lnc.tile([128, D], F32)
                    nc.sync.dma_start(bet_bc[:], beta_in.ap().to_broadcast((128, D)))

                    def outproj_group(qc, stl, mh, a_prev):
                        """One [128 rows x 512 cols] out-proj partial for chunk qc."""
                        rloc = slice(128 * stl, 128 * (stl + 1))
                        ms = slice(512 * mh, 512 * (mh + 1))
                        ps = aps.tile([128, 512], F32, name="py",
                                      tag=f"acc{mh}")
                        for j in range(ET):
                            nc.tensor.matmul(ps[:], a_prev[:, j, rloc],
                                             wo_sb[:, j, ms],
                                             start=(j == 0), stop=(j == ET - 1))
                        y_sb = ysb.tile([128, 512], BF16, name="y_sb", tag="y_sb")
                        # bo/2 folded here (host halves bo) so LN skips its add
                        nc.vector.tensor_add(y_sb[:], ps[:], bo_bc[:, ms])
                        grow = 512 * qc + 128 * stl
                        bi = grow // 256
                        nc.sync.dma_start(
                            y_part[bi][grow - 256 * bi:grow - 256 * bi + 128, ms],
                            y_sb[:])

                    # rs_block only fires the collective; the LN input DMAs
                    # are issued by prefetch() at a step where the RS wire is
                    # provably complete, so neither the in-order sync queue
                    # (HOL on the RS-done wait) nor the DVE queue (LN add
                    # waiting on a late DMA) ever blocks on the collective.
                    ln_in = {}

                    def rs_block(bi):
                        nc.gpsimd.collective_compute(
                            "ReduceScatter", mybir.AluOpType.add,
                            replica_groups=RG,
                            ins=[y_part[bi].opt()], outs=[rs_out[bi].opt()])

                    def prefetch(bi):
                        tb = lni.tile([128, D], BF16, name="tb", tag="tb")
                        nc.sync.dma_start(tb[:], rs_out[bi][:, :])
                        r = lni.tile([128, D], F32, name="r", tag="r")
                        nc.sync.dma_start(
                            r[:], xr_in.ap()[128 * bi:128 * (bi + 1), :])
                        ln_in[bi] = (tb, r)

                    def ln_block(bi, use_act=False):
                        """Residual + LayerNorm for this core's 128-row shard
                        of block bi. bo/2 pre-folded into the partials."""
                        tb, r = ln_in.pop(bi)
                        t = ln.tile([128, D], F32, name="t", tag="t")
                        nc.vector.tensor_add(t[:], r[:], tb[:])
                        vv = ln.tile([128, 1], F32, name="vv", tag="vv")
                        # stats stay on DVE: an ACT accum_out variant was
                        # tried and regressed -- the per-tile ACT<->DVE
                        # ping-pong costs more in cross-engine semaphore
                        # latency than the bn_stats work saved
                        stats = ln.tile([128, 2, 6], F32, name="stats",
                                        tag="stats")
                        tv = t[:].rearrange("p (a b) -> p a b", a=2)
                        for sub in range(2):
                            nc.vector.bn_stats(stats[:, sub, :],
                                               tv[:, sub, :])
                        mv = ln.tile([128, 2], F32, name="mv", tag="mv")
                        nc.vector.bn_aggr(mv[:], stats[:])
                        mean_ap = mv[:, 0:1]
                        nc.vector.tensor_scalar_add(vv[:], mv[:, 1:2], EPS)
                        # rstd = 1/sqrt(var+eps) on DVE (Newton from 1/v
                        # seed) so ACT runs exp only -> one table load ever
                        rstd = ln.tile([128, 1], F32, name="rstd", tag="rstd")
                        nc.vector.reciprocal(rstd[:], vv[:])
                        yt = ln.tile([128, 1], F32, name="yt", tag="yt")
                        for _ in range(2):
                            nc.vector.tensor_mul(yt[:], rstd[:], rstd[:])
                            nc.vector.tensor_mul(yt[:], yt[:], vv[:])
                            nc.vector.tensor_scalar(
                                yt[:], yt[:], -0.5, 1.5,
                                mybir.AluOpType.mult, mybir.AluOpType.add)
                            nc.vector.tensor_mul(rstd[:], rstd[:], yt[:])
                        o = t  # in-place: (t-mean)*rstd overwrites t (SBUF)
                        if use_act:
                            # drain only: apply (t-mean)*rstd on the (by
                            # then idle) ACT engine as
                            # Identity(rstd*t + (-mean*rstd)); Identity is
                            # in exp's table set so no reload. In-loop
                            # blocks keep this on DVE -- ACT is the phase
                            # B bottleneck.
                            nb = ln.tile([128, 1], F32, name="nb", tag="nb")
                            nc.vector.tensor_scalar(
                                nb[:], mean_ap, rstd[:], -1.0,
                                mybir.AluOpType.mult, mybir.AluOpType.mult)
                            nc.scalar.activation(
                                o[:], t[:],
                                mybir.ActivationFunctionType.Identity,
                                bias=nb[:], scale=rstd[:])
                        else:
                            nc.vector.tensor_scalar(
                                o[:], t[:], mean_ap, rstd[:],
                                mybir.AluOpType.subtract, mybir.AluOpType.mult)
                        nc.vector.tensor_mul(o[:], o[:], gam_bc[:])
                        nc.vector.tensor_add(o[:], o[:], bet_bc[:])
                        nc.sync.dma_start(
                            y_out.ap()[128 * bi:128 * (bi + 1), :], o[:])

                    # Flat software pipeline over all (qc, j, kti) steps with
                    # scores emitted TWO steps ahead: S(i+1) completes a full
                    # exp-period before exp(i+1) needs it, so the exp stream
                    # never waits on a PE round trip (was ~200ns/exp). PV(i-1)
                    # runs in exp(i)'s shadow. sps triple-buffers (6 banks);
                    # out-proj groups time-share the accumulator banks at
                    # j-boundaries (epilogue has just released them).
                    steps = [(qc, j, kti)
                             for qc in range(QC) for j in range(ET)
                             for kti in range(ST)]
                    sp_t, pp_t, accs_map, a_map = {}, {}, {}, {}

                    def emit_scores(i):
                        if i >= len(steps):
                            return
                        sqc, sj, skti = steps[i]
                        ks = slice(128 * skti, 128 * (skti + 1))
                        sqs = slice(512 * sqc, 512 * (sqc + 1))
                        sp = sps.tile([128, 1024], F32, name="sp", tag="sp")
                        for h01 in range(2):
                            rows = slice(64 * h01, 64 * (h01 + 1))
                            nc.tensor.matmul(sp[:, 512 * h01:512 * (h01 + 1)],
                                             kt[rows, sj, ks], qt[rows, sj, sqs],
                                             start=True, stop=True)
                        sp_t[i] = sp

                    def emit_pv(i):
                        pqc, pj, pkti = steps[i]
                        if pkti == 0:
                            accs_map[(pqc, pj)] = [
                                aps.tile([DEPTH + 1, 512], F32, name=f"acc{h}",
                                         tag=f"acc{h}")
                                for h in range(2)]
                        accs = accs_map[(pqc, pj)]
                        pp = pp_t.pop(i)
                        for h01 in range(2):
                            nc.tensor.matmul(accs[h01][:],
                                             vt[:, pkti, 2 * pj + h01, :],
                                             pp[:, 512 * h01:512 * (h01 + 1)],
                                             start=(pkti == 0),
                                             stop=(pkti == ST - 1))

                    def emit_epilogue(qc, j, a_t):
                        accs = accs_map.pop((qc, j))
                        # both acc-release copies first: frees both PSUM
                        # accumulator banks (out-proj groups wait on them via
                        # tag WAR) before the slow denominator DMA round
                        # trips, which then overlap h0/h1. The DRAM bounce is
                        # deliberate: a [1,512] DVE reciprocal costs 3.2us on
                        # one lane; bounced to a [64,8] scatter it costs 160ns
                        # and the DVE queue stays free.
                        acc_sbs = []
                        for h01 in range(2):
                            acc_sb = ep3.tile([DEPTH + 1, 512], F32, name="acc_sb",
                                              tag="acc_sb")
                            nc.vector.tensor_copy(acc_sb[:], accs[h01][:])
                            acc_sbs.append(acc_sb)
                        for h01 in range(2):
                            idx = (qc * ET + j) * 2 + h01
                            acc_sb = acc_sbs[h01]
                            nc.sync.dma_start(den_d[idx],
                                              acc_sb[DEPTH:DEPTH + 1, :])
                            rin = ep3.tile([64, 8], F32, name="rin", tag="rin")
                            nc.sync.dma_start(rin[:], den_d[idx].rearrange(
                                "a (p f) -> (a p) f", p=64))
                            nc.vector.reciprocal(rin[:], rin[:])
                            nc.sync.dma_start(rec_d[idx], rin[:])
                            rbc = ep3.tile([64, 512], F32, name="rbc", tag="rbc")
                            rsrc = rec_d[idx]
                            nc.sync.dma_start(
                                rbc[:],
                                bass.AP(tensor=rsrc.tensor, offset=rsrc.offset,
                                        ap=[[0, 64], [1, 512]]))
                            if h01 == 0:
                                nc.vector.tensor_mul(a_t[0:64, j, :],
                                                     acc_sb[0:DEPTH, :], rbc[:])
                            else:
                                nrm = ep3.tile([64, 512], F32R, name="nrm", tag="nrm")
                                nc.vector.tensor_mul(nrm[:], acc_sb[0:DEPTH, :],
                                                     rbc[:])
                                nc.sync.dma_start(a_t[64:128, j, :], nrm[:])

                    a_prev = None
                    fired, prefd = [], []   # (block, step index) queues
                    emit_scores(0)
                    emit_scores(1)
                    for i, (qc, j, kti) in enumerate(steps):
                        pp = psb.tile([128, 1024], F32R, name="pp", tag="pp")
                        nc.scalar.activation(pp[:], sp_t.pop(i)[:],
                                             mybir.ActivationFunctionType.Exp,
                                             scale=0.125)
                        pp_t[i] = pp
                        if i >= 1 and (i - 1) in pp_t:
                            emit_pv(i - 1)
                        if i + 2 not in sp_t:
                            emit_scores(i + 2)
                    # staged LN pipeline in quiet mid-loop slots, away
                    # from the boundary's epilogue/out-proj/RS pile-up:
                    # prefetch >=8 steps after the RS fired (wire done),
                    # LN compute >=4 steps after the prefetch (DMA done)
                        # prefetch only when the RS is PROVABLY complete
                        # (fire + pair skew + wire < 55 steps): a tb DMA
                        # waiting on a live collective would head-of-line
                        # block the sync queue and stall the epilogue chain
                        if kti == 8 and fired and i - fired[0][1] >= 70:
                            bi = fired.pop(0)[0]
                            prefetch(bi)
                            prefd.append((bi, i))
                        if kti == 4 and prefd and i - prefd[0][1] >= 6:
                            ln_block(prefd.pop(0)[0])
                        if kti == ST - 1:
                            emit_pv(i)
                            if j == 0:
                                a_map[qc] = atcp.tile([128, ET, 512], F32R,
                                                      name="a_t", tag="a_t")
                            emit_epilogue(qc, j, a_map[qc])
                            # pre-emit the next j's second scores tile so it
                            # sits ahead of the out-proj burst on the PE queue
                            # (same exp(i) gate as PV(i), so no extra stall)
                            if i + 3 < len(steps):
                                emit_scores(i + 3)
                            # boundary pieces: out-proj of qc-1 front-loaded
                            # [3,3,2,0] across the j-boundaries so both RS
                            # blocks fire a half-chunk early and the CC is
                            # idle when the drain's RS6/RS7 arrive
                            if qc >= 1:
                                for gi in ([0, 1, 2], [3, 4, 5], [6, 7], [])[j]:
                                    outproj_group(qc - 1, gi // 2, gi % 2,
                                                  a_prev)
                                if j in (1, 2):
                                    bi = 2 * (qc - 1) + (j - 1)
                                    rs_block(bi)
                                    fired.append((bi, i))
                            if j == ET - 1:
                                a_prev = a_map.pop(qc)
                    # drain: out-proj + RS for the last chunk, then ALL
                    # remaining LNs strictly after the out-proj adds so a
                    # late RS never head-of-line blocks the DVE queue ahead
                    # of out-proj work
                    outproj_group(QC - 1, 0, 0, a_prev)
                    outproj_group(QC - 1, 0, 1, a_prev)
                    outproj_group(QC - 1, 1, 0, a_prev)
                    outproj_group(QC - 1, 1, 1, a_prev)
                    rs_block(6)
                    outproj_group(QC - 1, 2, 0, a_prev)
                    outproj_group(QC - 1, 2, 1, a_prev)
                    outproj_group(QC - 1, 3, 0, a_prev)
                    outproj_group(QC - 1, 3, 1, a_prev)
                    rs_block(7)
                    for bi, _ in list(prefd):
                        ln_block(bi, use_act=True)
                    for bi in fired:
                        prefetch(bi[0])
                        ln_block(bi[0], use_act=True)
                    prefetch(6)
                    ln_block(6, use_act=True)
                    prefetch(7)
                    ln_block(7, use_act=True)

    nc.compile()
    return nc


def kernel(inputs, Wq, bq, Wk, bk, Wv, bv, Wo, bo, gamma, beta):
    if "nc" not in _CACHE:
        _CACHE["nc"] = _build()
    nc = _CACHE["nc"]

    inputs = np.ascontiguousarray(np.asarray(inputs, dtype=np.float32))
    Wq = np.asarray(Wq, np.float32); Wk = np.asarray(Wk, np.float32)
    Wv = np.asarray(Wv, np.float32); Wo = np.asarray(Wo, np.float32)
    bq = np.asarray(bq, np.float32); bk = np.asarray(bk, np.float32)
    bv = np.asarray(bv, np.float32); bo = np.asarray(bo, np.float32)
    gamma = np.asarray(gamma, np.float32); beta = np.asarray(beta, np.float32)

    in_maps = []
    for c in range(8):
        b, hf = c // 2, c % 2
        es = slice(EL * hf, EL * (hf + 1))
        bqk = np.concatenate([bq[es].reshape(ET, 128).T, bk[es].reshape(ET, 128).T],
                             axis=1)
        # this core's owned 128-row shard of each 256-row RS block
        xr = np.concatenate(
            [inputs[b][256 * i + 128 * hf:256 * i + 128 * (hf + 1)]
             for i in range(NBLK)], axis=0)
        in_maps.append({
            "x": inputs[b],
            "xr": np.ascontiguousarray(xr),
            "wq": np.ascontiguousarray(Wq[:, es]),
            "wk": np.ascontiguousarray(Wk[:, es]),
            "wv": np.ascontiguousarray(Wv[:, es]),
            "wo": np.ascontiguousarray(Wo[es, :]),
            "bqk": np.ascontiguousarray(bqk),
            "bv": bv[es].reshape(1, EL).copy(),
            "bo": (bo / 2.0).reshape(1, D).copy(),
            "gamma": gamma.reshape(1, D).copy(),
            "beta": beta.reshape(1, D).copy(),
        })

    global _LAST_IN_MAPS
    _LAST_IN_MAPS = in_maps
    res = run_bass_kernel_spmd(nc, in_maps, core_ids=list(range(8)))

    out = np.empty((B, S, D), dtype=np.float32)
    for c in range(8):
        b, hf = c // 2, c % 2
        yc = res.results[c]["y"]
        for i in range(NBLK):
            out[b, 256 * i + 128 * hf:256 * i + 128 * (hf + 1)] = \
                yc[128 * i:128 * (i + 1)]
    return out
